# revision 1
# baseline (speedup 1.0000x reference)
"""Multi-head self-attention (full-embed, no head split) on 8 Trainium2 cores.

Sharding: data-parallel over (batch=4) x (query-half=2) = 8 cores.
Each core computes Q projection for its 1024 query rows, K/V projections for
the full 2048-row sequence of its batch (duplicated across the core pair),
scores^T [k, q] -> exp -> (weights @ V) -> output projection for its rows.

All matmuls run as float32r (TF32-like, full PE rate); accumulation is fp32.
Layout choices:
  - x is passed transposed per core as xt [E, S], with the core's query half
    permuted to the front (softmax over k is permutation-invariant as long as
    V uses the same k order, which it does).
  - Q^T, K^T are produced in [f, s] layout so the scores matmul needs no
    transposes; scores^T is [k, q] so AV needs no transpose of the softmax
    weights; the softmax denominator Z is computed as a [1, q] row via a
    ones-column matmul and broadcast to [128, q] via a K=1 ones-row matmul.
  - V (natural [s, f] layout) is spilled to DRAM and streamed back during AV
    to stay within SBUF.
"""
import sys

sys.path.insert(0, '/opt/trn_rl_repo')

import numpy as np

import concourse.bass as bass
import concourse.bacc as bacc
import concourse.tile as tile
import concourse.mybir as mybir
from concourse import bass_utils

F32 = mybir.dt.float32
F32R = mybir.dt.float32r
AF = mybir.ActivationFunctionType

N_CORES = 8
B, S, E = 4, 2048, 1024
SH = S // 2          # per-core query rows
P = 128
EO = E // P          # 8 contraction chunks
FO = E // P          # 8 output-feature chunks
KO = S // P          # 16 key chunks
QB = 512             # q block (PSUM free dim)
NQB = SH // QB       # 2 q blocks per core
SCALE = 1.0 / np.sqrt(np.float32(E))

_CACHE = {}


def build_nc(loop_iters=None):
    """Build + compile the Bass module. loop_iters wraps the whole body in a
    hardware loop (used only for timing amplification by test harnesses)."""
    nc = bacc.Bacc("TRN2", target_bir_lowering=False, debug=False,
                   enable_asserts=False, num_devices=N_CORES)

    xt_ap = nc.dram_tensor("xt", [E, S], F32R, kind="ExternalInput").ap()
    wq_ap = nc.dram_tensor("wq", [FO, P, EO, P], F32R, kind="ExternalInput").ap()
    wk_ap = nc.dram_tensor("wk", [FO, P, EO, P], F32R, kind="ExternalInput").ap()
    wv_ap = nc.dram_tensor("wv", [2, P, EO, 512], F32R, kind="ExternalInput").ap()
    wo_ap = nc.dram_tensor("wo", [2, P, EO, 512], F32R, kind="ExternalInput").ap()
    bqr_ap = nc.dram_tensor("bqr", [P, FO], F32, kind="ExternalInput").ap()
    bkr_ap = nc.dram_tensor("bkr", [P, FO], F32, kind="ExternalInput").ap()
    bv_ap = nc.dram_tensor("bv", [E], F32, kind="ExternalInput").ap()
    bo_ap = nc.dram_tensor("bo", [E], F32, kind="ExternalInput").ap()
    ones_ap = nc.dram_tensor("ones", [P, P], F32R, kind="ExternalInput").ap()
    out_ap = nc.dram_tensor("out", [SH, E], F32, kind="ExternalOutput").ap()

    def bcast_ap(src, n):
        return bass.AP(tensor=src.tensor, offset=src.offset,
                       ap=[[0, P]] + list(src.ap))

    with tile.TileContext(nc) as tc:
        persist = tc.alloc_tile_pool(name="persist", bufs=1)
        dramp = tc.alloc_tile_pool(name="dramp", bufs=1, space="DRAM")

        def body():
            qt_sb = persist.tile([P, FO, SH], F32R, name="qt_sb")
            kt_sb = persist.tile([P, FO, S], F32R, name="kt_sb")
            bqr_sb = persist.tile([P, FO], F32, name="bqr_sb")
            bkr_sb = persist.tile([P, FO], F32, name="bkr_sb")
            bo_sb = persist.tile([P, E], F32, name="bo_sb")
            ones_sb = persist.tile([P, P], F32R, name="ones_sb")
            nc.sync.dma_start(out=bqr_sb, in_=bqr_ap)
            nc.sync.dma_start(out=bkr_sb, in_=bkr_ap)
            nc.gpsimd.dma_start(out=bo_sb, in_=bcast_ap(bo_ap, P))
            nc.sync.dma_start(out=ones_sb, in_=ones_ap)

            v_dram = dramp.tile([KO, 2, P, 512], F32R, name="v_dram")

            psA = tc.alloc_tile_pool(name="psA", bufs=1, space="PSUM")
            psB = tc.alloc_tile_pool(name="psB", bufs=1, space="PSUM")
            xtp = tc.alloc_tile_pool(name="xtp", bufs=1)

            xt_sb = xtp.tile([P, EO, S], F32R, name="xt_sb")
            for eo in range(EO):
                nc.sync.dma_start(out=xt_sb[:, eo, :],
                                  in_=xt_ap[eo * P:(eo + 1) * P, :])

            # ---- Phase 1a: Q^T (first SH cols) and K^T (full) projections
            wqk = tc.alloc_tile_pool(name="wqk", bufs=1)
            for fo in range(FO):
                wq_t = wqk.tile([P, EO, P], F32R, tag="wq", bufs=2, name="wq_t")
                nc.sync.dma_start(out=wq_t, in_=wq_ap[fo])
                wk_t = wqk.tile([P, EO, P], F32R, tag="wk", bufs=2, name="wk_t")
                nc.sync.dma_start(out=wk_t, in_=wk_ap[fo])
                psq = [psA.tile([P, 512], F32, tag="psq", bufs=2, name=f"psq{st}")
                       for st in range(2)]
                psk = [psA.tile([P, 512], F32, tag="psk", bufs=4, name=f"psk{st}")
                       for st in range(4)]
                for eo in range(EO):
                    st_first, st_last = (eo == 0), (eo == EO - 1)
                    for st in range(2):
                        nc.tensor.matmul(psq[st], lhsT=wq_t[:, eo, :],
                                         rhs=xt_sb[:, eo, st * 512:(st + 1) * 512],
                                         start=st_first, stop=st_last)
                    for st in range(4):
                        nc.tensor.matmul(psk[st], lhsT=wk_t[:, eo, :],
                                         rhs=xt_sb[:, eo, st * 512:(st + 1) * 512],
                                         start=st_first, stop=st_last)
                for st in range(2):
                    nc.scalar.activation(out=qt_sb[:, fo, st * 512:(st + 1) * 512],
                                         in_=psq[st], func=AF.Identity,
                                         bias=bqr_sb[:, fo:fo + 1], scale=1.0)
                for st in range(4):
                    nc.scalar.activation(out=kt_sb[:, fo, st * 512:(st + 1) * 512],
                                         in_=psk[st], func=AF.Identity,
                                         bias=bkr_sb[:, fo:fo + 1], scale=1.0)
            wqk.release()

            # ---- Phase 1b: V (natural [s, f]) -> DRAM spill, in f halves
            vpool = tc.alloc_tile_pool(name="vpool", bufs=1)
            bv_sb = vpool.tile([P, E], F32, name="bv_sb")
            nc.gpsimd.dma_start(out=bv_sb, in_=bcast_ap(bv_ap, P))
            for ft in range(2):
                wv_t = vpool.tile([P, EO, 512], F32R, tag="wv", bufs=1, name="wv_t")
                nc.sync.dma_start(out=wv_t, in_=wv_ap[ft])
                for so in range(KO):
                    psv = psB.tile([P, 512], F32, tag="psv", bufs=2, name="psv")
                    for eo in range(EO):
                        nc.tensor.matmul(psv, lhsT=xt_sb[:, eo, so * P:(so + 1) * P],
                                         rhs=wv_t[:, eo, :],
                                         start=(eo == 0), stop=(eo == EO - 1))
                    vst = vpool.tile([P, 512], F32R, tag="vst", bufs=2, name="vst")
                    with nc.allow_low_precision(reason="V feeds fp32r AV matmul"):
                        nc.vector.tensor_add(out=vst, in0=psv,
                                             in1=bv_sb[:, ft * 512:(ft + 1) * 512])
                    nc.sync.dma_start(out=v_dram[so, ft], in_=vst)
            vpool.release()
            xtp.release()
            psB.release()
            psA.release()

            # ---- Phase 2: per q-block attention + output projection
            ps_s = tc.alloc_tile_pool(name="ps_s", bufs=1, space="PSUM")
            ps_z = tc.alloc_tile_pool(name="ps_z", bufs=1, space="PSUM")
            ps_a = tc.alloc_tile_pool(name="ps_a", bufs=1, space="PSUM")
            ps_o = tc.alloc_tile_pool(name="ps_o", bufs=1, space="PSUM")
            blk = tc.alloc_tile_pool(name="blk", bufs=1)

            for qb in range(NQB):
                q0 = qb * QB
                exp_sb = blk.tile([P, KO, QB], F32R, tag="exp", bufs=1, name="exp_sb")
                # scores^T -> exp
                for ko in range(KO):
                    pss = ps_s.tile([P, QB], F32, tag="pss", bufs=2, name="pss")
                    for fo in range(FO):
                        nc.tensor.matmul(pss, lhsT=kt_sb[:, fo, ko * P:(ko + 1) * P],
                                         rhs=qt_sb[:, fo, q0:q0 + QB],
                                         start=(fo == 0), stop=(fo == FO - 1))
                    nc.scalar.activation(out=exp_sb[:, ko, :], in_=pss,
                                         func=AF.Exp, scale=float(SCALE))
                # Z row = column sums of exp (over all k)
                psz = ps_z.tile([P, QB], F32, tag="psz", bufs=2, name="psz")
                for ko in range(KO):
                    nc.tensor.matmul(psz[:1, :], lhsT=ones_sb[:, 0:1],
                                     rhs=exp_sb[:, ko, :],
                                     start=(ko == 0), stop=(ko == KO - 1))
                zinv = blk.tile([1, QB], F32R, tag="zinv", bufs=2, name="zinv")
                with nc.allow_low_precision(reason="zinv feeds fp32r matmul"):
                    nc.vector.reciprocal(out=zinv[:1, :], in_=psz[:1, :])
                # broadcast zinv over partitions via K=1 ones-row matmul
                psb = ps_z.tile([P, QB], F32, tag="psz", bufs=2, name="psb")
                nc.tensor.matmul(psb, lhsT=ones_sb[:1, :], rhs=zinv[:1, :],
                                 start=True, stop=True)
                zb_sb = blk.tile([P, QB], F32, tag="zb", bufs=2, name="zb_sb")
                nc.vector.tensor_copy(out=zb_sb, in_=psb)

                # AV: attn^T[e, q] = sum_k V[k, e] * exp[k, q], normalized
                attn_sb = blk.tile([P, EO, QB], F32R, tag="attn", bufs=1,
                                   name="attn_sb")
                for ft in range(2):
                    for g in range(2):
                        psp = [ps_a.tile([P, QB], F32, tag="psa", bufs=2,
                                         name=f"psa{j}") for j in range(2)]
                        for ko in range(KO):
                            vch = blk.tile([P, 256], F32R, tag="vch", bufs=4,
                                           name="vch")
                            nc.sync.dma_start(
                                out=vch,
                                in_=v_dram[ko, ft][:, g * 256:(g + 1) * 256])
                            for j in range(2):
                                nc.tensor.matmul(psp[j],
                                                 lhsT=vch[:, j * P:(j + 1) * P],
                                                 rhs=exp_sb[:, ko, :],
                                                 start=(ko == 0),
                                                 stop=(ko == KO - 1))
                        eo0 = ft * 4 + g * 2
                        for j in range(2):
                            with nc.allow_low_precision(
                                    reason="attn feeds fp32r out-proj matmul"):
                                nc.vector.tensor_mul(out=attn_sb[:, eo0 + j, :],
                                                     in0=psp[j], in1=zb_sb)

                # output projection + bias, DMA out
                for ft in range(2):
                    wo_t = blk.tile([P, EO, 512], F32R, tag="wo", bufs=1,
                                    name="wo_t")
                    nc.sync.dma_start(out=wo_t, in_=wo_ap[ft])
                    for qo in range(4):
                        pso = ps_o.tile([P, 512], F32, tag="pso", bufs=2,
                                        name="pso")
                        for eo in range(EO):
                            nc.tensor.matmul(pso,
                                             lhsT=attn_sb[:, eo, qo * P:(qo + 1) * P],
                                             rhs=wo_t[:, eo, :],
                                             start=(eo == 0), stop=(eo == EO - 1))
                        ost = blk.tile([P, 512], F32, tag="ost", bufs=2,
                                       name="ost")
                        nc.vector.tensor_add(out=ost, in0=pso,
                                             in1=bo_sb[:, ft * 512:(ft + 1) * 512])
                        row = q0 + qo * P
                        nc.sync.dma_start(
                            out=out_ap[row:row + P, ft * 512:(ft + 1) * 512],
                            in_=ost)

            blk.release()
            ps_o.release()
            ps_a.release()
            ps_z.release()
            ps_s.release()

        if loop_iters is None:
            body()
        else:
            with tc.For_i(0, loop_iters):
                body()

        dramp.release()
        persist.release()

    nc.compile()
    return nc


def _prep_shared(Wq, bq, Wk, bk, Wv, bv, Wo, bo):
    def chunk_w(W, free):
        wT = np.ascontiguousarray(W.T)  # [e_in, f_out]
        n = E // free
        return np.ascontiguousarray(
            wT.reshape(EO, P, n, free).transpose(2, 1, 0, 3))

    return {
        "wq": chunk_w(Wq, P),
        "wk": chunk_w(Wk, P),
        "wv": chunk_w(Wv, 512),
        "wo": chunk_w(Wo, 512),
        "bqr": np.ascontiguousarray(bq.reshape(FO, P).T),
        "bkr": np.ascontiguousarray(bk.reshape(FO, P).T),
        "bv": np.ascontiguousarray(bv),
        "bo": np.ascontiguousarray(bo),
        "ones": np.ones((P, P), dtype=np.float32),
    }


def make_in_maps(x, Wq, bq, Wk, bk, Wv, bv, Wo, bo):
    shared = _prep_shared(Wq, bq, Wk, bk, Wv, bv, Wo, bo)
    in_maps = []
    for c in range(N_CORES):
        b, h = c // 2, c % 2
        xt = np.asarray(x[b]).T  # [E, S]
        if h == 0:
            xt_p = np.ascontiguousarray(xt)
        else:
            xt_p = np.ascontiguousarray(
                np.concatenate([xt[:, SH:], xt[:, :SH]], axis=1))
        m = {"xt": xt_p}
        m.update(shared)
        in_maps.append(m)
    return in_maps


def kernel(x, Wq, bq, Wk, bk, Wv, bv, Wo, bo):
    x = np.asarray(x, dtype=np.float32)
    args = [np.asarray(a, dtype=np.float32)
            for a in (Wq, bq, Wk, bk, Wv, bv, Wo, bo)]
    if "nc" not in _CACHE:
        _CACHE["nc"] = build_nc()
    nc = _CACHE["nc"]
    in_maps = make_in_maps(x, *args)
    res = bass_utils.run_bass_kernel_spmd(nc, in_maps,
                                          core_ids=list(range(N_CORES)))
    out = np.empty((B, S, E), dtype=np.float32)
    for c in range(N_CORES):
        b, h = c // 2, c % 2
        out[b, h * SH:(h + 1) * SH, :] = res.results[c]["out"]
    return out
